# revision 48
# baseline (speedup 1.0000x reference)
"""Multi-head attention (B=2, N=2048, C=1024, H=16, D=64) on 8 TRN2 NeuronCores.

Sharding: core c = (batch b = c//4) x (head-group g = c%4 -> heads 4g..4g+3).
Data parallel on B, tensor parallel on heads.  After each head's softmax
normalization, the [64, cw] head-output is AllGathered (fp16) within the
4-core batch group; each core then runs the full-K (1024) out-projection
for its 256-channel output slice locally -- no reduce, no post-projection
collective, and the gathers overlap the attention pipeline.

The projections are software-pipelined into the attention stream: only the
k-projection (m2) and the first half of the q-projection (m0 @ n0,n1) run
before head 0 starts; the remaining qk projection chunks, the whole
v-projection and the previous chunk's out-projection quadrants are folded
into the attention pipeline's PE slack (the inner loop is ACT/exp-bound).

Everything on device stays transposed ([channel, position]); the host
pre-transposes inputs and post-transposes the output.
"""

import numpy as np

import concourse.bacc as bacc
import concourse.tile as tile
import concourse.mybir as mybir
from concourse.bass_utils import run_bass_kernel_spmd

B, N, C, H = 2, 2048, 1024, 16
D = C // H          # 64
HL = H // 4         # 4 heads per core
CL = HL * D         # 256 local channels
N_CORES = 8
GROUPS = [[0, 1, 2, 3], [4, 5, 6, 7]]

F32 = mybir.dt.float32
BF16 = mybir.dt.float16
BF = np.float16

KC = C // 128       # 8  K-chunks of the input channel dim
NI = N // 512       # 4  512-wide i-chunks
NJ = N // 128       # 16 128-row j-chunks


def build_kernel(n_cores=N_CORES, groups=GROUPS):
    group_size = len(groups[0])
    out_rows = C // group_size          # 256 output channels per core

    nc = bacc.Bacc("TRN2", target_bir_lowering=False, debug=False,
                   num_devices=n_cores)

    # all inputs are host-packed into 128-partition-major panels so each
    # loads in one full-bandwidth DMA (per-transfer setup dominates at
    # smaller sizes): x as (half, kc, 1024) columns, weights kc-major
    xT = nc.declare_dram_parameter("xT", [128, 2 * KC * 1024], BF16,
                                   isOutput=False)
    cos2 = nc.declare_dram_parameter("cos2", [128, N], BF16, isOutput=False)
    sin2s = nc.declare_dram_parameter("sin2s", [128, N], BF16, isOutput=False)
    wqkT = nc.declare_dram_parameter("wqkT", [128, KC * 512], BF16,
                                     isOutput=False)
    bqk = nc.declare_dram_parameter("bqk", [128, 4], F32, isOutput=False)
    wvT = nc.declare_dram_parameter("wvT", [128, KC * CL], BF16,
                                    isOutput=False)
    # out-projection weights, rows ordered (head_local, rank, d) to match
    # the AllGather concat order; cols = this core's 256 output channels
    wprojT = nc.declare_dram_parameter("wprojT", [128, KC * out_rows], BF16,
                                       isOutput=False)
    beff = nc.declare_dram_parameter("beff", [128, 2], F32, isOutput=False)
    out = nc.declare_dram_parameter("out", [out_rows, N], F32, isOutput=True)

    with tile.TileContext(nc) as tc:
        with tc.tile_pool(name="dram", bufs=1, space="DRAM") as dram, \
             tc.tile_pool(name="sbuf", bufs=1) as sb, \
             tc.tile_pool(name="psum", bufs=1, space="PSUM") as ps:

            # tile for clock-warming matmuls (see _warm_pe)
            warm = sb.tile([128, 128], BF16, name="warm", tag="warm")
            nc.vector.memset(warm[:], 0.001)

            def _warm_pe(tag, n):
                # short matmuls alternating two PSUM tiles: keeps the PE's
                # activity monitor busy so the clock gate stays at full rate
                wps = [ps.tile([128, 64], F32, name=f"warmp{tag}_{a}",
                               tag="sc", bufs=3) for a in range(2)]
                for r in range(n):
                    nc.tensor.matmul(wps[r % 2][:], warm[:], warm[:, :64],
                                     start=True, stop=True)

            _warm_pe("s", 16)

            # ---- input DMA: wqk first, then x in n-major column slices so
            # the kc-outer qk matmul chain for chunk n can start as soon as
            # that chunk's 8 column slices land ----
            # x is loaded in two [*, 1024] column halves (2KB lines keep the
            # DMA at full efficiency): the first half unblocks head 0's
            # whole q range and its first 8 score chunks; the second half
            # streams under head 0's pipeline.
            #
            # The "scalar" DMA queue is the ACT engine's instruction queue:
            # any trigger parked there blocks the evictions/exp stream
            # behind it (head-of-line, ring-gated by transfer completions).
            # So ACT only carries early-critical triggers; the bulk goes on
            # sync, and the second-half x triggers for ACT are emitted after
            # the pre-attention evictions (see below).
            bqk_sb = sb.tile([128, 4], F32, name="bqk_sb", tag="bqk")
            nc.sync.dma_start(bqk_sb[:], bqk.ap())
            wqk_t = sb.tile([128, KC * 512], BF16, name="wqk_t", tag="wqk")
            nc.scalar.dma_start(wqk_t[:], wqkT.ap())
            # first x half ahead of everything else (it gates the whole
            # front-end), quartered so the kc-outer qk chain can trickle
            x_sb = sb.tile([128, 2 * KC * 1024], BF16, name="x_sb", tag="x")
            for qtr in range(4):
                eng = nc.sync if qtr % 2 == 0 else nc.scalar
                eng.dma_start(x_sb[:, 2048 * qtr:2048 * (qtr + 1)],
                              xT.ap()[:, 2048 * qtr:2048 * (qtr + 1)])
            cos_sb = sb.tile([128, N], BF16, name="cos_sb", tag="cos_sb")
            sin_sb = sb.tile([128, N], BF16, name="sin_sb", tag="sin_sb")
            nc.sync.dma_start(cos_sb[:], cos2.ap())
            nc.scalar.dma_start(sin_sb[:], sin2s.ap())
            wv_t = sb.tile([128, KC * CL], BF16, name="wv_t", tag="wv")
            nc.sync.dma_start(wv_t[:], wvT.ap())
            nc.sync.dma_start(x_sb[:, 8192:12288], xT.ap()[:, 8192:12288])
            nc.scalar.dma_start(x_sb[:, 12288:16384], xT.ap()[:, 12288:16384])
            wproj_t = sb.tile([128, KC * out_rows], BF16, name="wproj_t",
                              tag="wproj")
            nc.sync.dma_start(wproj_t[:], wprojT.ap())
            beff_sb = sb.tile([128, 2], F32, name="beff_sb", tag="beff")
            nc.sync.dma_start(beff_sb[:], beff.ap())

            def xap(kc, c0, c1):
                # x panel columns: 8192*half + 1024*kc + (c % 1024)
                nn = c0 // 1024
                base = 8192 * nn + 1024 * kc + (c0 - 1024 * nn)
                return x_sb[:, base:base + (c1 - c0)]

            def wqk_ap(kc, m):
                return wqk_t[:, 512 * kc + 128 * m:512 * kc + 128 * (m + 1)]

            def wv_ap(kc):
                return wv_t[:, CL * kc:CL * (kc + 1)]

            def wproj_ap(kc, mc):
                return wproj_t[:, out_rows * kc + 128 * mc:
                               out_rows * kc + 128 * (mc + 1)]

            # warm up the collective path during the preamble so the first
            # real gather doesn't absorb the first-collective cost
            agw_in = dram.tile([64, 8], BF16, name="agw_in", tag="agw_in")
            agw_out = dram.tile([64 * group_size, 8], BF16, name="agw_out",
                                tag="agw_out")
            agw_sb = sb.tile([64, 8], BF16, name="agw_sb", tag="agw_sb")
            nc.vector.memset(agw_sb[:], 0.0)
            nc.sync.dma_start(agw_in[:], agw_sb[:])
            nc.gpsimd.collective_compute(
                "AllGather", mybir.AluOpType.bypass, replica_groups=groups,
                ins=[agw_in[:]], outs=[agw_out[:]])

            # ---- qk projection + RoPE, per (m, n-chunk) job ----
            # chunk m rows: m=0:[q_h0,q_h1] m=1:[q_h2,q_h3] m=2:[k_h0,k_h1] m=3:[k_h2,k_h3]
            # so q and k of head h sit at the same partition offset 64*(h%2).
            # k of each head lands in its own zero-padded [128, N] tile so the
            # scores matmul can contract over K=128 (16-bit matmuls run at
            # half rate for K=64 -- zero rows buy back the full rate).
            k_t = []      # 4 tiles: k_h at rows 64*(h%2), zeros elsewhere
            for h in range(4):
                kt = sb.tile([128, N], BF16, name=f"ktile{h}", tag=f"ktile{h}")
                z = slice(0, 64) if h % 2 == 1 else slice(64, 128)
                # gpsimd: these 1.8us memsets would clog the DVE queue
                # ahead of the RoPE chain that gates the first exp
                nc.gpsimd.memset(kt[z, :], 0.0)
                k_t.append(kt)
            swap_mask = [i ^ 1 for i in range(32)]
            qks_t = [sb.tile([128, N], BF16, name=f"qks{m}", tag=f"qks{m}")
                     for m in range(4)]
            q_r = [sb.tile([128, N], BF16, name=f"qkr{m}", tag=f"qkr{m}")
                   for m in range(2)]

            def qk_mm(nn, m):
                # accumulate one 1024-wide projection half over the 8 kc
                # chunks (single PSUM acc, two 512 column groups)
                acc = ps.tile([128, 1024], F32, name=f"qa{nn}_{m}", tag="sc",
                              bufs=3)
                for kc in range(KC):
                    for q in range(2):
                        nc.tensor.matmul(
                            acc[:, 512 * q:512 * (q + 1)],
                            wqk_ap(kc, m),
                            xap(kc, 1024 * nn + 512 * q,
                                1024 * nn + 512 * (q + 1)),
                            start=(kc == 0), stop=(kc == KC - 1))
                return acc

            def qk_fin(nn, m, acc, act=True):
                # bias-add eviction + RoPE.  When folded this runs two jc
                # slots after the matmuls so the ACT/DVE work never waits on
                # in-flight PE instructions (head-of-line on the exp stream).
                nsl = slice(1024 * nn, 1024 * (nn + 1))
                qks = qks_t[m]
                if act:
                    nc.scalar.activation(
                        qks[:, nsl], acc[:],
                        mybir.ActivationFunctionType.Identity,
                        bias=bqk_sb[:, m:m + 1])
                else:
                    nc.vector.tensor_scalar_add(qks[:, nsl], acc[:],
                                                bqk_sb[:, m:m + 1])
                # RoPE: qk' = qks*cos2 + shift(qks)*sin2s
                # (pair-swap of adjacent partitions via DVE stream shuffle)
                shf = sb.tile([128, 1024], BF16, name=f"shf{nn}_{m}",
                              tag="shf", bufs=2)
                nc.vector.stream_shuffle(shf[:], qks[:, nsl], swap_mask)
                t2 = sb.tile([128, 1024], BF16, name=f"rtmp{nn}_{m}",
                             tag="ropetmp", bufs=2)
                nc.vector.tensor_mul(t2[:], shf[:], sin_sb[:, nsl])
                if m < 2:
                    qkr = q_r[m]
                    nc.vector.tensor_mul(qkr[:, nsl], qks[:, nsl],
                                         cos_sb[:, nsl])
                    nc.vector.tensor_add(qkr[:, nsl], qkr[:, nsl], t2[:])
                else:
                    t1 = sb.tile([128, 1024], BF16, name=f"rtc{nn}_{m}",
                                 tag="ropetc", bufs=2)
                    nc.vector.tensor_mul(t1[:], qks[:, nsl], cos_sb[:, nsl])
                    h0, h1 = 2 * (m - 2), 2 * (m - 2) + 1
                    nc.vector.tensor_add(k_t[h0][0:64, nsl], t1[0:64, :],
                                         t2[0:64, :])
                    nc.vector.tensor_add(k_t[h1][64:128, nsl], t1[64:128, :],
                                         t2[64:128, :])

            def qk_job(nn, m, act=True):
                qk_fin(nn, m, qk_mm(nn, m), act)

            # pre-attention: k-proj and q-proj only for what head 0's first
            # 8 score chunks touch (first column half); the rest is folded
            # into the attention pipeline as the second x half lands
            qk_job(0, 2)
            qk_job(0, 0)

            # ---- v projection (natural [j, ch] layout, ones col per head) ----
            vaug = [None] * NJ

            def vproj_mm(jp):
                pvs = [ps.tile([128, CL], F32, name=f"pv{2*jp+a}", tag="sc",
                               bufs=3) for a in range(2)]
                for kc in range(KC):
                    for a in range(2):
                        jc = 2 * jp + a
                        nc.tensor.matmul(
                            pvs[a][:],
                            xap(kc, 128 * jc, 128 * (jc + 1)),
                            wv_ap(kc),
                            start=(kc == 0), stop=(kc == KC - 1))
                return pvs

            def vproj_fin(jp, pvs, act=True):
                for a in range(2):
                    jc = 2 * jp + a
                    va = sb.tile([128, HL * (D + 1)], BF16, name=f"vaug{jc}",
                                 tag=f"vaug{jc}")
                    nc.vector.memset(va[:, D::D + 1], 1.0)
                    if act:
                        nc.scalar.activation(
                            va.rearrange("p (h e) -> p h e", e=D + 1)[:, :, 0:D],
                            pvs[a].rearrange("p (h e) -> p h e", e=D)[:, :, :],
                            mybir.ActivationFunctionType.Copy)
                    else:
                        nc.vector.tensor_copy(
                            va.rearrange("p (h e) -> p h e", e=D + 1)[:, :, 0:D],
                            pvs[a].rearrange("p (h e) -> p h e", e=D)[:, :, :])
                    vaug[jc] = va

            # per-partition bias AP used to shift scores before fp16 exp
            eshift = sb.tile([128, 1], F32, name="eshift", tag="eshift")
            nc.vector.memset(eshift[:], -16.0)
            # K=1 ones row used to broadcast denominators across partitions
            ones64 = sb.tile([1, 64], BF16, name="ones64", tag="ones64")
            nc.vector.memset(ones64[:], 1.0)

            # ---- attention, per i-chunk; per-head AllGather of the
            # normalized output; remaining projections and the previous
            # chunk's out-projection folded into the head pipelines ----
            chunks = [(0, 1024), (1024, 1024)]
            n_chunks = len(chunks)

            ag_in = [[dram.tile([64, cw], BF16, name=f"agin{ih}_{hl}",
                                tag=f"agin{ih}_{hl}")
                      for hl in range(4)] for ih, (i0, cw) in enumerate(chunks)]
            ag_out = [[dram.tile([64 * group_size, cw], BF16,
                                 name=f"agout{ih}_{hl}", tag=f"agout{ih}_{hl}")
                       for hl in range(4)] for ih, (i0, cw) in enumerate(chunks)]

            # AGs fired so far, in order; readbacks are emitted two head-slots
            # after the AG fires so a pending readback on the gpsimd queue
            # never sits between two AG triggers (that would serialize the
            # collective stream: AG k+1 couldn't trigger until AG k finished)
            ag_fired = []
            rb_state = {"done": 0}

            def drain_readbacks(limit=1):
                while len(ag_fired) - rb_state["done"] > limit:
                    fih, fhl, fcw = ag_fired[rb_state["done"]]
                    emit_readback(fih, fhl, fcw)
                    rb_state["done"] += 1

            def finalize_head(ih, hl, oacc, cw):
                # normalize: o[:, i] / den[i].  Broadcast den across
                # partitions with a K=1 matmul, then reciprocal+mul on 64
                # partitions; gather the per-head result across the group.
                den = sb.tile([1, cw], BF16, name=f"den{ih}_{hl}",
                              tag="den", bufs=2)
                nc.vector.tensor_copy(den[:], oacc[64:65, :])
                rb = ps.tile([64, cw], F32, name=f"rb{ih}_{hl}",
                             tag="sc", bufs=3)
                for q in range(cw // 512):
                    nc.tensor.matmul(rb[:, 512 * q:512 * (q + 1)],
                                     ones64[:],
                                     den[:, 512 * q:512 * (q + 1)],
                                     start=True, stop=True)
                rr = sb.tile([64, cw], F32, name=f"rr{ih}_{hl}", tag="rr",
                             bufs=2)
                nc.vector.reciprocal_approx_fast(rr[:], rb[:])
                oh = sb.tile([64, cw], BF16, name=f"oh{ih}_{hl}", tag="oh",
                             bufs=2)
                nc.vector.tensor_mul(oh[:], oacc[0:64, :], rr[:])
                nc.sync.dma_start(ag_in[ih][hl][:], oh[:])
                nc.gpsimd.collective_compute(
                    "AllGather", mybir.AluOpType.bypass,
                    replica_groups=groups,
                    ins=[ag_in[ih][hl][:]],
                    outs=[ag_out[ih][hl][:]])
                ag_fired.append((ih, hl, cw))

            # gathered o readback tiles, kc = hl*2 + half (row order matches
            # wprojT's (head_local, rank, d) ordering)
            o_rb = [[None] * KC for _ in range(n_chunks)]

            def emit_readback(ih, hl, cw):
                for half in range(2):
                    t = sb.tile([128, cw], BF16, name=f"orb{ih}_{hl}_{half}",
                                tag="orb", bufs=12)
                    nc.gpsimd.dma_start(
                        t[:], ag_out[ih][hl][128 * half:128 * (half + 1), :])
                    o_rb[ih][2 * hl + half] = t

            def proj_mm(ih, mc, n2, i0, pp, kcs, start, stop):
                isl = slice(512 * n2, 512 * (n2 + 1))
                for kc in kcs:
                    nc.tensor.matmul(
                        pp[:],
                        wproj_ap(kc, mc),
                        o_rb[ih][kc][:, isl],
                        start=start and kc == kcs[0],
                        stop=stop and kc == kcs[-1])

            def proj_evict(ih, mc, n2, i0, pp):
                fo = sb.tile([128, 512], F32, name=f"fo{ih}_{mc}_{n2}",
                             tag="fo", bufs=4)
                nc.vector.tensor_scalar_add(fo[:], pp[:],
                                            beff_sb[:, mc:mc + 1])
                nc.sync.dma_start(
                    out.ap()[128 * mc:128 * (mc + 1),
                             i0 + 512 * n2:i0 + 512 * (n2 + 1)], fo[:])

            def emit_proj_job(ih, mc, n2, i0, cw):
                pp = ps.tile([128, 512], F32, name=f"pp{ih}_{mc}_{n2}",
                             tag="sc", bufs=3)
                proj_mm(ih, mc, n2, i0, pp, list(range(KC)), True, True)
                return pp

            def fold_proj(ih, hl, jc, job):
                fold2(ih, hl, jc, 2, lambda: emit_proj_job(*job),
                      lambda pp: proj_evict(job[0], job[1], job[2], job[3],
                                            pp))

            # fold schedule: (ih, hl, jc) -> list of thunks to emit inside
            # the attention pipeline at that point (PE slack).  Folded jobs
            # are two-stage: matmuls at jc, eviction/RoPE at jc+2, so the
            # ACT/DVE side never blocks the exp stream on in-flight matmuls
            folds = {}

            def add_fold(ih, hl, jc, fn):
                folds.setdefault((ih, hl, jc), []).append(fn)

            def fold2(ih, hl, jc, gap, mm_fn, fin_fn):
                # two-stage: matmuls at jc, eviction at jc+gap (so the
                # ACT/DVE side never waits on in-flight matmuls).  gap is
                # bounded by the sc-slot rotation: a fold's PSUM acc slot is
                # reassigned by the 3rd-next sc-tag tile, and the eviction
                # read must be emitted before that reassigning tile.
                box = {}
                add_fold(ih, hl, jc, lambda: box.__setitem__("a", mm_fn()))
                add_fold(ih, hl, min(jc + gap, NJ - 1),
                         lambda: fin_fn(box["a"]))

            def fold_qk(ih, hl, jc, nn, m):
                fold2(ih, hl, jc, 2, lambda: qk_mm(nn, m),
                      lambda acc: qk_fin(nn, m, acc, act=True))

            def fold_vp(ih, hl, jc, jp):
                # two pv tiles: slot reassigned one creation earlier -> gap 1
                fold2(ih, hl, jc, 1, lambda: vproj_mm(jp),
                      lambda pvs: vproj_fin(jp, pvs, act=True))

            # the first half of the v-projection runs in the preamble
            # (the PE is DMA-gated idle there and wv/x arrive in time);
            # head 0 absorbs the second-half k-proj (m2 @ nn1, ahead of the
            # jc8+ groups that read those k columns) and the rest of the
            # v-projection; the second x half streams in underneath
            for jp in range(4):
                vproj_fin(jp, vproj_mm(jp), act=False)
            fold_qk(0, 0, 4, 1, 2)
            fold_vp(0, 0, 6, 4)
            fold_vp(0, 0, 8, 5)
            fold_vp(0, 0, 10, 6)
            fold_vp(0, 0, 12, 7)
            # k-proj heads 2,3 (m3): k_t[2] is read from head 2 on, so both
            # halves land in head 1.  q-proj m1 @ first half must be emitted
            # before head 2 (its scores read q_r[1][:, 0:1024] from jc0);
            # m1/m0 second halves before chunk 1 reads them.
            fold_qk(0, 1, 2, 0, 3)
            fold_qk(0, 1, 8, 1, 3)
            fold_qk(0, 1, 13, 0, 1)
            fold_qk(0, 2, 2, 1, 0)
            fold_qk(0, 3, 2, 1, 1)

            for ih, (i0, cw) in enumerate(chunks):
                ns = cw // 512
                # out-projection quadrants of the previous chunk, folded
                # into heads 2,3 (the last gather lands mid-head-1)
                if ih > 0:
                    pi0, pcw = chunks[ih - 1]
                    jobs = [(ih - 1, mc, n2, pi0, pcw)
                            for mc in range(out_rows // 128)
                            for n2 in range(pcw // 512)]
                    fold_proj(ih, 2, 2, jobs[0])
                    fold_proj(ih, 2, 8, jobs[1])
                    fold_proj(ih, 3, 2, jobs[2])
                    fold_proj(ih, 3, 8, jobs[3])
                pending = None
                for hl in range(4):
                    qT = q_r[hl // 2]
                    kT = k_t[hl]
                    oacc = ps.tile([65, cw], F32, name=f"oacc{ih}_{hl}",
                                   tag="oacc", bufs=1)
                    exs = []

                    def emit_o(jc, oacc=oacc, exs=exs, hl=hl):
                        for q in range(ns):
                            nc.tensor.matmul(
                                oacc[:, 512 * q:512 * (q + 1)],
                                vaug[jc][:, (D + 1) * hl:(D + 1) * (hl + 1)],
                                exs[jc][:, 512 * q:512 * (q + 1)],
                                start=(jc == 0), stop=(jc == NJ - 1))

                    scs = {}

                    def emit_sc(jc, hl=hl, kT=kT, qT=qT, scs=scs):
                        sc = ps.tile([128, cw], F32, name=f"sc{ih}_{hl}_{jc}",
                                     tag="sc", bufs=3)
                        for q in range(ns):
                            nc.tensor.matmul(
                                sc[:, 512 * q:512 * (q + 1)],
                                kT[:, 128 * jc:128 * (jc + 1)],
                                qT[:, i0 + 512 * q:i0 + 512 * (q + 1)],
                                start=True, stop=True)
                        scs[jc] = sc

                    # scores run two j-chunks ahead of the exp stream so a
                    # folded projection never leaves ACT without buffered work
                    emit_sc(0)
                    emit_sc(1)
                    for jc in range(NJ):
                        ex = sb.tile([128, cw], BF16, name=f"ex{ih}_{hl}_{jc}",
                                     tag="ex", bufs=5)
                        # bias shifts all scores so fp16 exp can't overflow
                        # (softmax is shift-invariant, cancels in num/den)
                        nc.scalar.activation(ex[:], scs.pop(jc),
                                             mybir.ActivationFunctionType.Exp,
                                             scale=float(1.0 / np.sqrt(D)),
                                             bias=eshift[:])
                        exs.append(ex)
                        # software pipeline: the previous head's
                        # normalization chain is deferred into this head's
                        # pipeline so the exp stream never pauses at head
                        # boundaries
                        if jc == 1 and pending is not None:
                            finalize_head(*pending)
                            pending = None
                        # folds before the next score emission: a fold's
                        # finish stage reads a PSUM acc whose sc-tag slot is
                        # reassigned by the tile created three positions
                        # later -- the read must be emitted before that
                        for fn in folds.pop((ih, hl, jc), ()):
                            fn()
                        if jc + 2 < NJ:
                            emit_sc(jc + 2)
                        if jc >= 2:
                            # o-matmuls lag TWO j-chunks: the PE queue is
                            # in-order, so a shorter lag couples PE progress
                            # to the exp stream and leaves no buffered exp
                            # work when a folded projection occupies the PE
                            emit_o(jc - 2)
                    emit_o(NJ - 2)
                    emit_o(NJ - 1)
                    pending = (ih, hl, oacc, cw)
                    # emit readbacks for gathers fired two head-slots ago
                    drain_readbacks(limit=1)
                # keep the PE clock-gate warm through the last head's
                # normalization chain so the tail starts at full rate
                _warm_pe(f"p{ih}", 12)
                if ih < n_chunks - 1:
                    finalize_head(*pending)
                    pending = None

            # ---- tail ----
            # finalize + gather the last head like any other, then the out-
            # projection quadrants: kc 0..5 (heads 0-2, gathers already
            # landed) run during the final gather; kc 6,7 finish after
            drain_readbacks(limit=0)
            finalize_head(*pending)
            pending = None
            drain_readbacks(limit=0)
            li0, lcw = chunks[-1]
            tail_jobs = [(n_chunks - 1, mc, n2, li0, lcw)
                         for mc in range(out_rows // 128)
                         for n2 in range(lcw // 512)]
            pps = []
            for j, (ihx, mc, n2, i0, cw) in enumerate(tail_jobs):
                pp = ps.tile([128, 512], F32, name=f"tpp{mc}_{n2}",
                             tag="sc" if j < 3 else "oacc",
                             bufs=3 if j < 3 else 1)
                proj_mm(ihx, mc, n2, i0, pp, list(range(6)), True, False)
                pps.append(pp)
            for j, (ihx, mc, n2, i0, cw) in enumerate(tail_jobs):
                proj_mm(ihx, mc, n2, i0, pps[j], [6, 7], False, True)
                proj_evict(ihx, mc, n2, i0, pps[j])

    nc.compile()
    return nc


def shard_inputs(x, rope, w_qkv, b_qkv, w_proj, b_proj,
                 n_cores=N_CORES, group_size=4):
    """Per-core input maps. Host-side transposes/casts are part of sharding."""
    out_rows = C // group_size
    # fold the v-bias through the projection into an effective output bias
    b_v = b_qkv[2 * C:3 * C]
    b_eff = (b_proj + b_v @ w_proj.T).astype(np.float32)   # [C]

    in_maps = []
    for c in range(n_cores):
        b = (c // group_size) % B
        g = c % group_size
        heads = range(HL * g, HL * g + HL)

        xTb = np.ascontiguousarray(x[b].T).astype(BF)            # [C, N]
        # pack into the [128, (half, kc, 1024)] panel the kernel expects
        x_r = np.ascontiguousarray(
            xTb.reshape(KC, 128, 2, 1024).transpose(1, 2, 0, 3)
            .reshape(128, 2 * KC * 1024))

        cosT = rope[b].T[:D, :]                                   # [64, N]
        sinT = rope[b].T[D:, :]
        cos2 = np.vstack([cosT, cosT]).astype(BF)                 # [128, N]
        sgn = np.where(np.arange(128) % 2 == 0, -1.0, 1.0)[:, None]
        sin2s = (np.vstack([sinT, sinT]) * sgn).astype(BF)        # [128, N]

        # qk weight rows ordered [q_h0..q_h3, k_h0..k_h3]
        qk_rows = []
        bqk_rows = []
        for h in heads:
            qk_rows.append(w_qkv[D * h:D * (h + 1), :])           # q rows
            bqk_rows.append(b_qkv[D * h:D * (h + 1)])
        for h in heads:
            qk_rows.append(w_qkv[C + D * h:C + D * (h + 1), :])   # k rows
            bqk_rows.append(b_qkv[C + D * h:C + D * (h + 1)])
        wqk = np.vstack(qk_rows)                                  # [512, C]
        wqkT = np.ascontiguousarray(wqk.T).astype(BF)             # [C, 512]
        wqk_r = np.ascontiguousarray(
            wqkT.reshape(KC, 128, 512).transpose(1, 0, 2)
            .reshape(128, KC * 512))
        bqk_v = np.ascontiguousarray(
            np.concatenate(bqk_rows).astype(np.float32).reshape(4, 128).T)

        h0 = HL * g
        wv = w_qkv[2 * C + D * h0:2 * C + D * h0 + CL, :]          # [256, C]
        wvT = np.ascontiguousarray(wv.T).astype(BF)                # [C, 256]
        wv_r = np.ascontiguousarray(
            wvT.reshape(KC, 128, CL).transpose(1, 0, 2)
            .reshape(128, KC * CL))

        # out-projection: this core owns output channels
        # [out_rows*g, out_rows*(g+1)).  The contraction rows are ordered
        # (head_local hl, rank r, d) to match the per-head AllGather layout
        # (gather hl concatenates rank blocks of 64 rows).
        osl = slice(out_rows * g, out_rows * (g + 1))
        wp_rows = []
        for hl in range(HL):
            for r in range(group_size):
                gh = HL * r + hl                   # global head of (r, hl)
                wp_rows.append(w_proj[osl, D * gh:D * (gh + 1)].T)  # [64,256]
        wprojT = np.ascontiguousarray(np.vstack(wp_rows)).astype(BF)  # [C,256]
        wproj_r = np.ascontiguousarray(
            wprojT.reshape(KC, 128, out_rows).transpose(1, 0, 2)
            .reshape(128, KC * out_rows))

        beff_shard = np.ascontiguousarray(
            b_eff[osl].astype(np.float32).reshape(2, 128).T)

        in_maps.append({
            "xT": x_r, "cos2": cos2, "sin2s": sin2s,
            "wqkT": wqk_r, "bqk": bqk_v, "wvT": wv_r,
            "wprojT": wproj_r, "beff": beff_shard,
        })
    return in_maps


def assemble(results, n_cores=N_CORES, group_size=4):
    out_rows = C // group_size
    out = np.empty((B, N, C), dtype=np.float32)
    for c in range(n_cores):
        b = (c // group_size) % B
        g = c % group_size
        outT_shard = results[c]["out"]                 # [out_rows, N] f32
        out[b, :, out_rows * g:out_rows * (g + 1)] = outT_shard.T
    return out


_NC_CACHE = {}


def _get_nc():
    if "nc" not in _NC_CACHE:
        _NC_CACHE["nc"] = build_kernel()
    return _NC_CACHE["nc"]


def _run(inputs, trace=False, tmpdir=None):
    nc = _get_nc()
    inputs = {k: np.asarray(v) for k, v in inputs.items()}
    in_maps = shard_inputs(**inputs)
    res = run_bass_kernel_spmd(nc, in_maps, core_ids=list(range(N_CORES)),
                               trace=trace, tmpdir=tmpdir)
    return assemble(res.results), res


def kernel(**inputs):
    out, _ = _run(inputs)
    return out


# revision 49
# speedup vs baseline: 1.0562x; 1.0562x over previous
"""Multi-head attention (B=2, N=2048, C=1024, H=16, D=64) on 8 TRN2 NeuronCores.

Sharding: core c = (batch b = c//4) x (head-group g = c%4 -> heads 4g..4g+3).
Data parallel on B, tensor parallel on heads.  After each head's softmax
normalization, the [64, cw] head-output is AllGathered (fp16) within the
4-core batch group; each core then runs the full-K (1024) out-projection
for its 256-channel output slice locally -- no reduce, no post-projection
collective, and the gathers overlap the attention pipeline.

The projections are software-pipelined into the attention stream: only the
k-projection (m2) and the first half of the q-projection (m0 @ n0,n1) run
before head 0 starts; the remaining qk projection chunks, the whole
v-projection and the previous chunk's out-projection quadrants are folded
into the attention pipeline's PE slack (the inner loop is ACT/exp-bound).

Everything on device stays transposed ([channel, position]); the host
pre-transposes inputs and post-transposes the output.
"""

import numpy as np

import concourse.bacc as bacc
import concourse.tile as tile
import concourse.mybir as mybir
from concourse.bass_utils import run_bass_kernel_spmd

B, N, C, H = 2, 2048, 1024, 16
D = C // H          # 64
HL = H // 4         # 4 heads per core
CL = HL * D         # 256 local channels
N_CORES = 8
GROUPS = [[0, 1, 2, 3], [4, 5, 6, 7]]

F32 = mybir.dt.float32
BF16 = mybir.dt.float16
BF = np.float16

KC = C // 128       # 8  K-chunks of the input channel dim
NI = N // 512       # 4  512-wide i-chunks
NJ = N // 128       # 16 128-row j-chunks


def build_kernel(n_cores=N_CORES, groups=GROUPS):
    group_size = len(groups[0])
    out_rows = C // group_size          # 256 output channels per core

    nc = bacc.Bacc("TRN2", target_bir_lowering=False, debug=False,
                   num_devices=n_cores)

    # all inputs are host-packed into 128-partition-major panels so each
    # loads in one full-bandwidth DMA (per-transfer setup dominates at
    # smaller sizes): x as (half, kc, 1024) columns, weights kc-major
    xT = nc.declare_dram_parameter("xT", [128, 2 * KC * 1024], BF16,
                                   isOutput=False)
    cos2 = nc.declare_dram_parameter("cos2", [128, N], BF16, isOutput=False)
    sin2s = nc.declare_dram_parameter("sin2s", [128, N], BF16, isOutput=False)
    wqkT = nc.declare_dram_parameter("wqkT", [128, KC * 512], BF16,
                                     isOutput=False)
    bqk = nc.declare_dram_parameter("bqk", [128, 4], F32, isOutput=False)
    wvT = nc.declare_dram_parameter("wvT", [128, KC * CL], BF16,
                                    isOutput=False)
    # out-projection weights, rows ordered (head_local, rank, d) to match
    # the AllGather concat order; cols = this core's 256 output channels
    wprojT = nc.declare_dram_parameter("wprojT", [128, KC * out_rows], BF16,
                                       isOutput=False)
    beff = nc.declare_dram_parameter("beff", [128, 2], F32, isOutput=False)
    out = nc.declare_dram_parameter("out", [out_rows, N], F32, isOutput=True)

    with tile.TileContext(nc) as tc:
        with tc.tile_pool(name="dram", bufs=1, space="DRAM") as dram, \
             tc.tile_pool(name="sbuf", bufs=1) as sb, \
             tc.tile_pool(name="psum", bufs=1, space="PSUM") as ps:

            # tile for clock-warming matmuls (see _warm_pe)
            warm = sb.tile([128, 128], BF16, name="warm", tag="warm")
            nc.vector.memset(warm[:], 0.001)

            def _warm_pe(tag, n):
                # short matmuls alternating two PSUM tiles: keeps the PE's
                # activity monitor busy so the clock gate stays at full rate
                wps = [ps.tile([128, 64], F32, name=f"warmp{tag}_{a}",
                               tag="sc", bufs=3) for a in range(2)]
                for r in range(n):
                    nc.tensor.matmul(wps[r % 2][:], warm[:], warm[:, :64],
                                     start=True, stop=True)

            _warm_pe("s", 16)

            # ---- input DMA: wqk first, then x in n-major column slices so
            # the kc-outer qk matmul chain for chunk n can start as soon as
            # that chunk's 8 column slices land ----
            # x is loaded in two [*, 1024] column halves (2KB lines keep the
            # DMA at full efficiency): the first half unblocks head 0's
            # whole q range and its first 8 score chunks; the second half
            # streams under head 0's pipeline.
            #
            # The "scalar" DMA queue is the ACT engine's instruction queue:
            # any trigger parked there blocks the evictions/exp stream
            # behind it (head-of-line, ring-gated by transfer completions).
            # So ACT only carries early-critical triggers; the bulk goes on
            # sync, and the second-half x triggers for ACT are emitted after
            # the pre-attention evictions (see below).
            bqk_sb = sb.tile([128, 4], F32, name="bqk_sb", tag="bqk")
            nc.sync.dma_start(bqk_sb[:], bqk.ap())
            wqk_t = sb.tile([128, KC * 512], BF16, name="wqk_t", tag="wqk")
            nc.scalar.dma_start(wqk_t[:], wqkT.ap())
            # first x half ahead of everything else (it gates the whole
            # front-end), quartered so the kc-outer qk chain can trickle
            x_sb = sb.tile([128, 2 * KC * 1024], BF16, name="x_sb", tag="x")
            for qtr in range(4):
                eng = nc.sync if qtr % 2 == 0 else nc.scalar
                eng.dma_start(x_sb[:, 2048 * qtr:2048 * (qtr + 1)],
                              xT.ap()[:, 2048 * qtr:2048 * (qtr + 1)])
            cos_sb = sb.tile([128, N], BF16, name="cos_sb", tag="cos_sb")
            sin_sb = sb.tile([128, N], BF16, name="sin_sb", tag="sin_sb")
            nc.sync.dma_start(cos_sb[:], cos2.ap())
            nc.scalar.dma_start(sin_sb[:], sin2s.ap())
            wv_t = sb.tile([128, KC * CL], BF16, name="wv_t", tag="wv")
            nc.sync.dma_start(wv_t[:], wvT.ap())
            nc.sync.dma_start(x_sb[:, 8192:12288], xT.ap()[:, 8192:12288])
            nc.scalar.dma_start(x_sb[:, 12288:16384], xT.ap()[:, 12288:16384])
            wproj_t = sb.tile([128, KC * out_rows], BF16, name="wproj_t",
                              tag="wproj")
            nc.sync.dma_start(wproj_t[:], wprojT.ap())
            beff_sb = sb.tile([128, 2], F32, name="beff_sb", tag="beff")
            nc.sync.dma_start(beff_sb[:], beff.ap())

            def xap(kc, c0, c1):
                # x panel columns: 8192*half + 1024*kc + (c % 1024)
                nn = c0 // 1024
                base = 8192 * nn + 1024 * kc + (c0 - 1024 * nn)
                return x_sb[:, base:base + (c1 - c0)]

            def wqk_ap(kc, m):
                return wqk_t[:, 512 * kc + 128 * m:512 * kc + 128 * (m + 1)]

            def wv_ap(kc):
                return wv_t[:, CL * kc:CL * (kc + 1)]

            def wproj_ap(kc, mc):
                return wproj_t[:, out_rows * kc + 128 * mc:
                               out_rows * kc + 128 * (mc + 1)]

            # warm up the collective path during the preamble so the first
            # real gather doesn't absorb the first-collective cost
            agw_in = dram.tile([64, 8], BF16, name="agw_in", tag="agw_in")
            agw_out = dram.tile([64 * group_size, 8], BF16, name="agw_out",
                                tag="agw_out")
            agw_sb = sb.tile([64, 8], BF16, name="agw_sb", tag="agw_sb")
            nc.vector.memset(agw_sb[:], 0.0)
            nc.sync.dma_start(agw_in[:], agw_sb[:])
            nc.gpsimd.collective_compute(
                "AllGather", mybir.AluOpType.bypass, replica_groups=groups,
                ins=[agw_in[:]], outs=[agw_out[:]])

            # ---- qk projection + RoPE, per (m, n-chunk) job ----
            # chunk m rows: m=0:[q_h0,q_h1] m=1:[q_h2,q_h3] m=2:[k_h0,k_h1] m=3:[k_h2,k_h3]
            # so q and k of head h sit at the same partition offset 64*(h%2).
            # k of each head lands in its own zero-padded [128, N] tile so the
            # scores matmul can contract over K=128 (16-bit matmuls run at
            # half rate for K=64 -- zero rows buy back the full rate).
            k_t = []      # 4 tiles: k_h at rows 64*(h%2), zeros elsewhere
            for h in range(4):
                kt = sb.tile([128, N], BF16, name=f"ktile{h}", tag=f"ktile{h}")
                z = slice(0, 64) if h % 2 == 1 else slice(64, 128)
                # gpsimd: these 1.8us memsets would clog the DVE queue
                # ahead of the RoPE chain that gates the first exp
                nc.gpsimd.memset(kt[z, :], 0.0)
                k_t.append(kt)
            swap_mask = [i ^ 1 for i in range(32)]
            qks_t = [sb.tile([128, N], BF16, name=f"qks{m}", tag=f"qks{m}")
                     for m in range(4)]
            q_r = [sb.tile([128, N], BF16, name=f"qkr{m}", tag=f"qkr{m}")
                   for m in range(2)]

            def qk_mm(nn, m):
                # accumulate one 1024-wide projection half over the 8 kc
                # chunks (single PSUM acc, two 512 column groups)
                acc = ps.tile([128, 1024], F32, name=f"qa{nn}_{m}", tag="sc",
                              bufs=3)
                for kc in range(KC):
                    for q in range(2):
                        nc.tensor.matmul(
                            acc[:, 512 * q:512 * (q + 1)],
                            wqk_ap(kc, m),
                            xap(kc, 1024 * nn + 512 * q,
                                1024 * nn + 512 * (q + 1)),
                            start=(kc == 0), stop=(kc == KC - 1))
                return acc

            def qk_fin(nn, m, acc, act=True):
                # bias-add eviction + RoPE.  When folded this runs two jc
                # slots after the matmuls so the ACT/DVE work never waits on
                # in-flight PE instructions (head-of-line on the exp stream).
                nsl = slice(1024 * nn, 1024 * (nn + 1))
                qks = qks_t[m]
                if act:
                    nc.scalar.activation(
                        qks[:, nsl], acc[:],
                        mybir.ActivationFunctionType.Identity,
                        bias=bqk_sb[:, m:m + 1])
                else:
                    nc.vector.tensor_scalar_add(qks[:, nsl], acc[:],
                                                bqk_sb[:, m:m + 1])
                # RoPE: qk' = qks*cos2 + shift(qks)*sin2s
                # (pair-swap of adjacent partitions via DVE stream shuffle)
                shf = sb.tile([128, 1024], BF16, name=f"shf{nn}_{m}",
                              tag="shf", bufs=2)
                nc.vector.stream_shuffle(shf[:], qks[:, nsl], swap_mask)
                t2 = sb.tile([128, 1024], BF16, name=f"rtmp{nn}_{m}",
                             tag="ropetmp", bufs=2)
                nc.vector.tensor_mul(t2[:], shf[:], sin_sb[:, nsl])
                if m < 2:
                    qkr = q_r[m]
                    nc.vector.tensor_mul(qkr[:, nsl], qks[:, nsl],
                                         cos_sb[:, nsl])
                    nc.vector.tensor_add(qkr[:, nsl], qkr[:, nsl], t2[:])
                else:
                    t1 = sb.tile([128, 1024], BF16, name=f"rtc{nn}_{m}",
                                 tag="ropetc", bufs=2)
                    nc.vector.tensor_mul(t1[:], qks[:, nsl], cos_sb[:, nsl])
                    h0, h1 = 2 * (m - 2), 2 * (m - 2) + 1
                    nc.vector.tensor_add(k_t[h0][0:64, nsl], t1[0:64, :],
                                         t2[0:64, :])
                    nc.vector.tensor_add(k_t[h1][64:128, nsl], t1[64:128, :],
                                         t2[64:128, :])

            def qk_job(nn, m, act=True):
                qk_fin(nn, m, qk_mm(nn, m), act)

            # pre-attention: k-proj and q-proj only for what head 0's first
            # 8 score chunks touch (first column half); the rest is folded
            # into the attention pipeline as the second x half lands
            qk_job(0, 2)
            qk_job(0, 0)

            # ---- v projection (natural [j, ch] layout, ones col per head) ----
            vaug = [None] * NJ

            def vproj_mm(jp):
                pvs = [ps.tile([128, CL], F32, name=f"pv{2*jp+a}", tag="sc",
                               bufs=3) for a in range(2)]
                for kc in range(KC):
                    for a in range(2):
                        jc = 2 * jp + a
                        nc.tensor.matmul(
                            pvs[a][:],
                            xap(kc, 128 * jc, 128 * (jc + 1)),
                            wv_ap(kc),
                            start=(kc == 0), stop=(kc == KC - 1))
                return pvs

            def vproj_fin(jp, pvs, act=True):
                for a in range(2):
                    jc = 2 * jp + a
                    va = sb.tile([128, HL * (D + 1)], BF16, name=f"vaug{jc}",
                                 tag=f"vaug{jc}")
                    nc.vector.memset(va[:, D::D + 1], 1.0)
                    if act:
                        nc.scalar.activation(
                            va.rearrange("p (h e) -> p h e", e=D + 1)[:, :, 0:D],
                            pvs[a].rearrange("p (h e) -> p h e", e=D)[:, :, :],
                            mybir.ActivationFunctionType.Copy)
                    else:
                        nc.vector.tensor_copy(
                            va.rearrange("p (h e) -> p h e", e=D + 1)[:, :, 0:D],
                            pvs[a].rearrange("p (h e) -> p h e", e=D)[:, :, :])
                    vaug[jc] = va

            # per-partition bias AP used to shift scores before fp16 exp
            eshift = sb.tile([128, 1], F32, name="eshift", tag="eshift")
            nc.vector.memset(eshift[:], -16.0)
            # K=1 ones row used to broadcast denominators across partitions
            ones64 = sb.tile([1, 64], BF16, name="ones64", tag="ones64")
            nc.vector.memset(ones64[:], 1.0)

            # ---- attention, per i-chunk; per-head AllGather of the
            # normalized output; remaining projections and the previous
            # chunk's out-projection folded into the head pipelines ----
            chunks = [(0, 1024), (1024, 1024)]
            n_chunks = len(chunks)

            ag_in = [[dram.tile([64, cw], BF16, name=f"agin{ih}_{hl}",
                                tag=f"agin{ih}_{hl}")
                      for hl in range(4)] for ih, (i0, cw) in enumerate(chunks)]
            ag_out = [[dram.tile([64 * group_size, cw], BF16,
                                 name=f"agout{ih}_{hl}", tag=f"agout{ih}_{hl}")
                       for hl in range(4)] for ih, (i0, cw) in enumerate(chunks)]

            # AGs fired so far, in order; readbacks are emitted two head-slots
            # after the AG fires so a pending readback on the gpsimd queue
            # never sits between two AG triggers (that would serialize the
            # collective stream: AG k+1 couldn't trigger until AG k finished)
            ag_fired = []
            rb_state = {"done": 0}

            def drain_readbacks(limit=1):
                while len(ag_fired) - rb_state["done"] > limit:
                    fih, fhl, fcw = ag_fired[rb_state["done"]]
                    emit_readback(fih, fhl, fcw)
                    rb_state["done"] += 1

            def finalize_head(ih, hl, oacc, cw):
                # normalize: o[:, i] / den[i].  Broadcast den across
                # partitions with a K=1 matmul, then reciprocal+mul on 64
                # partitions; gather the per-head result across the group.
                den = sb.tile([1, cw], BF16, name=f"den{ih}_{hl}",
                              tag="den", bufs=2)
                nc.vector.tensor_copy(den[:], oacc[64:65, :])
                rb = ps.tile([64, cw], F32, name=f"rb{ih}_{hl}",
                             tag="sc", bufs=3)
                for q in range(cw // 512):
                    nc.tensor.matmul(rb[:, 512 * q:512 * (q + 1)],
                                     ones64[:],
                                     den[:, 512 * q:512 * (q + 1)],
                                     start=True, stop=True)
                rr = sb.tile([64, cw], F32, name=f"rr{ih}_{hl}", tag="rr",
                             bufs=2)
                nc.vector.reciprocal_approx_fast(rr[:], rb[:])
                oh = sb.tile([64, cw], BF16, name=f"oh{ih}_{hl}", tag="oh",
                             bufs=2)
                nc.vector.tensor_mul(oh[:], oacc[0:64, :], rr[:])
                nc.sync.dma_start(ag_in[ih][hl][:], oh[:])
                nc.gpsimd.collective_compute(
                    "AllGather", mybir.AluOpType.bypass,
                    replica_groups=groups,
                    ins=[ag_in[ih][hl][:]],
                    outs=[ag_out[ih][hl][:]])
                ag_fired.append((ih, hl, cw))

            # gathered o readback tiles, kc = hl*2 + half (row order matches
            # wprojT's (head_local, rank, d) ordering)
            o_rb = [[None] * KC for _ in range(n_chunks)]

            def emit_readback(ih, hl, cw):
                for half in range(2):
                    t = sb.tile([128, cw], BF16, name=f"orb{ih}_{hl}_{half}",
                                tag="orb", bufs=12)
                    nc.gpsimd.dma_start(
                        t[:], ag_out[ih][hl][128 * half:128 * (half + 1), :])
                    o_rb[ih][2 * hl + half] = t

            def proj_mm(ih, mc, n2, i0, pp, kcs, start, stop):
                isl = slice(512 * n2, 512 * (n2 + 1))
                for kc in kcs:
                    nc.tensor.matmul(
                        pp[:],
                        wproj_ap(kc, mc),
                        o_rb[ih][kc][:, isl],
                        start=start and kc == kcs[0],
                        stop=stop and kc == kcs[-1])

            def proj_evict(ih, mc, n2, i0, pp):
                fo = sb.tile([128, 512], F32, name=f"fo{ih}_{mc}_{n2}",
                             tag="fo", bufs=4)
                nc.vector.tensor_scalar_add(fo[:], pp[:],
                                            beff_sb[:, mc:mc + 1])
                nc.sync.dma_start(
                    out.ap()[128 * mc:128 * (mc + 1),
                             i0 + 512 * n2:i0 + 512 * (n2 + 1)], fo[:])

            def emit_proj_job(ih, mc, n2, i0, cw):
                pp = ps.tile([128, 512], F32, name=f"pp{ih}_{mc}_{n2}",
                             tag="sc", bufs=3)
                proj_mm(ih, mc, n2, i0, pp, list(range(KC)), True, True)
                return pp

            def fold_proj(ih, hl, jc, job):
                fold2(ih, hl, jc, 2, lambda: emit_proj_job(*job),
                      lambda pp: proj_evict(job[0], job[1], job[2], job[3],
                                            pp))

            # fold schedule: (ih, hl, jc) -> list of thunks to emit inside
            # the attention pipeline at that point (PE slack).  Folded jobs
            # are two-stage: matmuls at jc, eviction/RoPE at jc+2, so the
            # ACT/DVE side never blocks the exp stream on in-flight matmuls
            folds = {}

            def add_fold(ih, hl, jc, fn):
                folds.setdefault((ih, hl, jc), []).append(fn)

            def fold2(ih, hl, jc, gap, mm_fn, fin_fn):
                # two-stage: matmuls at jc, eviction at jc+gap (so the
                # ACT/DVE side never waits on in-flight matmuls).  gap is
                # bounded by the sc-slot rotation: a fold's PSUM acc slot is
                # reassigned by the 3rd-next sc-tag tile, and the eviction
                # read must be emitted before that reassigning tile.
                box = {}
                add_fold(ih, hl, jc, lambda: box.__setitem__("a", mm_fn()))
                add_fold(ih, hl, min(jc + gap, NJ - 1),
                         lambda: fin_fn(box["a"]))

            def qk_mm_half(nn, m, q, acc=None):
                # one 512-wide half of a qk projection job (a full 3.4us job
                # inserted at a single jc overruns the ~2 buffered exps the
                # scores-ahead pipeline provides; halves fit)
                if acc is None:
                    acc = ps.tile([128, 1024], F32, name=f"qa{nn}_{m}",
                                  tag="sc", bufs=3)
                for kc in range(KC):
                    nc.tensor.matmul(
                        acc[:, 512 * q:512 * (q + 1)],
                        wqk_ap(kc, m),
                        xap(kc, 1024 * nn + 512 * q,
                            1024 * nn + 512 * (q + 1)),
                        start=(kc == 0), stop=(kc == KC - 1))
                return acc

            def fold_qk(ih, hl, jc, nn, m):
                box = {}
                add_fold(ih, hl, jc,
                         lambda: box.__setitem__("a", qk_mm_half(nn, m, 0)))
                add_fold(ih, hl, min(jc + 1, NJ - 1),
                         lambda: qk_mm_half(nn, m, 1, box["a"]))
                add_fold(ih, hl, min(jc + 2, NJ - 1),
                         lambda: qk_fin(nn, m, box["a"], act=True))

            def fold_vp(ih, hl, jc, jp):
                # two pv tiles: slot reassigned one creation earlier -> gap 1
                fold2(ih, hl, jc, 1, lambda: vproj_mm(jp),
                      lambda pvs: vproj_fin(jp, pvs, act=True))

            # the first half of the v-projection runs in the preamble
            # (the PE is DMA-gated idle there and wv/x arrive in time);
            # head 0 absorbs the second-half k-proj (m2 @ nn1, ahead of the
            # jc8+ groups that read those k columns) and the rest of the
            # v-projection; the second x half streams in underneath
            for jp in range(4):
                vproj_fin(jp, vproj_mm(jp), act=False)
            fold_qk(0, 0, 4, 1, 2)
            fold_vp(0, 0, 6, 4)
            fold_vp(0, 0, 8, 5)
            fold_vp(0, 0, 10, 6)
            fold_vp(0, 0, 12, 7)
            # k-proj heads 2,3 (m3): k_t[2] is read from head 2 on, so both
            # halves land in head 1.  q-proj m1 @ first half must be emitted
            # before head 2 (its scores read q_r[1][:, 0:1024] from jc0);
            # m1/m0 second halves before chunk 1 reads them.
            fold_qk(0, 1, 2, 0, 3)
            fold_qk(0, 1, 8, 1, 3)
            fold_qk(0, 1, 13, 0, 1)
            fold_qk(0, 2, 2, 1, 0)
            fold_qk(0, 3, 2, 1, 1)

            for ih, (i0, cw) in enumerate(chunks):
                ns = cw // 512
                # out-projection quadrants of the previous chunk, folded
                # into heads 2,3 (the last gather lands mid-head-1)
                if ih > 0:
                    pi0, pcw = chunks[ih - 1]
                    jobs = [(ih - 1, mc, n2, pi0, pcw)
                            for mc in range(out_rows // 128)
                            for n2 in range(pcw // 512)]
                    add_fold(ih, 2, 1,
                             lambda: drain_readbacks(limit=len(ag_fired) - 4))
                    fold_proj(ih, 2, 2, jobs[0])
                    fold_proj(ih, 2, 8, jobs[1])
                    fold_proj(ih, 3, 2, jobs[2])
                    fold_proj(ih, 3, 8, jobs[3])
                pending = None
                for hl in range(4):
                    qT = q_r[hl // 2]
                    kT = k_t[hl]
                    oacc = ps.tile([65, cw], F32, name=f"oacc{ih}_{hl}",
                                   tag="oacc", bufs=1)
                    exs = []

                    def emit_o(jc, oacc=oacc, exs=exs, hl=hl):
                        for q in range(ns):
                            nc.tensor.matmul(
                                oacc[:, 512 * q:512 * (q + 1)],
                                vaug[jc][:, (D + 1) * hl:(D + 1) * (hl + 1)],
                                exs[jc][:, 512 * q:512 * (q + 1)],
                                start=(jc == 0), stop=(jc == NJ - 1))

                    scs = {}

                    def emit_sc(jc, hl=hl, kT=kT, qT=qT, scs=scs):
                        sc = ps.tile([128, cw], F32, name=f"sc{ih}_{hl}_{jc}",
                                     tag="sc", bufs=3)
                        for q in range(ns):
                            nc.tensor.matmul(
                                sc[:, 512 * q:512 * (q + 1)],
                                kT[:, 128 * jc:128 * (jc + 1)],
                                qT[:, i0 + 512 * q:i0 + 512 * (q + 1)],
                                start=True, stop=True)
                        scs[jc] = sc

                    # scores run two j-chunks ahead of the exp stream so a
                    # folded projection never leaves ACT without buffered work
                    emit_sc(0)
                    emit_sc(1)
                    for jc in range(NJ):
                        ex = sb.tile([128, cw], BF16, name=f"ex{ih}_{hl}_{jc}",
                                     tag="ex", bufs=5)
                        # bias shifts all scores so fp16 exp can't overflow
                        # (softmax is shift-invariant, cancels in num/den)
                        nc.scalar.activation(ex[:], scs.pop(jc),
                                             mybir.ActivationFunctionType.Exp,
                                             scale=float(1.0 / np.sqrt(D)),
                                             bias=eshift[:])
                        exs.append(ex)
                        # software pipeline: the previous head's
                        # normalization chain is deferred into this head's
                        # pipeline so the exp stream never pauses at head
                        # boundaries
                        if jc == 1 and pending is not None:
                            finalize_head(*pending)
                            pending = None
                        # folds before the next score emission: a fold's
                        # finish stage reads a PSUM acc whose sc-tag slot is
                        # reassigned by the tile created three positions
                        # later -- the read must be emitted before that
                        for fn in folds.pop((ih, hl, jc), ()):
                            fn()
                        if jc + 2 < NJ:
                            emit_sc(jc + 2)
                        if jc >= 2:
                            # o-matmuls lag TWO j-chunks: the PE queue is
                            # in-order, so a shorter lag couples PE progress
                            # to the exp stream and leaves no buffered exp
                            # work when a folded projection occupies the PE
                            emit_o(jc - 2)
                    emit_o(NJ - 2)
                    emit_o(NJ - 1)
                    pending = (ih, hl, oacc, cw)
                    # emit readbacks for gathers fired three head-slots
                    # ago: by then the gather is long done, so the readback
                    # never parks on the gpsimd queue ahead of the next
                    # gather triggers (which would serialize the stream)
                    drain_readbacks(limit=2)
                # keep the PE clock-gate warm through the last head's
                # normalization chain so the tail starts at full rate
                _warm_pe(f"p{ih}", 12)
                if ih < n_chunks - 1:
                    finalize_head(*pending)
                    pending = None

            # ---- tail ----
            # finalize + gather the last head like any other, then the out-
            # projection quadrants: kc 0..5 (heads 0-2, gathers already
            # landed) run during the final gather; kc 6,7 finish after
            # the last gather's trigger must be enqueued before any
            # still-pending readback (a readback parked on the gpsimd queue
            # blocks triggers behind it until its own gather finishes)
            finalize_head(*pending)
            pending = None
            drain_readbacks(limit=0)
            li0, lcw = chunks[-1]
            tail_jobs = [(n_chunks - 1, mc, n2, li0, lcw)
                         for mc in range(out_rows // 128)
                         for n2 in range(lcw // 512)]
            pps = []
            for j, (ihx, mc, n2, i0, cw) in enumerate(tail_jobs):
                pp = ps.tile([128, 512], F32, name=f"tpp{mc}_{n2}",
                             tag="sc" if j < 3 else "oacc",
                             bufs=3 if j < 3 else 1)
                proj_mm(ihx, mc, n2, i0, pp, list(range(6)), True, False)
                pps.append(pp)
            for j, (ihx, mc, n2, i0, cw) in enumerate(tail_jobs):
                proj_mm(ihx, mc, n2, i0, pps[j], [6, 7], False, True)
                proj_evict(ihx, mc, n2, i0, pps[j])

    nc.compile()
    return nc


def shard_inputs(x, rope, w_qkv, b_qkv, w_proj, b_proj,
                 n_cores=N_CORES, group_size=4):
    """Per-core input maps. Host-side transposes/casts are part of sharding."""
    out_rows = C // group_size
    # fold the v-bias through the projection into an effective output bias
    b_v = b_qkv[2 * C:3 * C]
    b_eff = (b_proj + b_v @ w_proj.T).astype(np.float32)   # [C]

    in_maps = []
    for c in range(n_cores):
        b = (c // group_size) % B
        g = c % group_size
        heads = range(HL * g, HL * g + HL)

        xTb = np.ascontiguousarray(x[b].T).astype(BF)            # [C, N]
        # pack into the [128, (half, kc, 1024)] panel the kernel expects
        x_r = np.ascontiguousarray(
            xTb.reshape(KC, 128, 2, 1024).transpose(1, 2, 0, 3)
            .reshape(128, 2 * KC * 1024))

        cosT = rope[b].T[:D, :]                                   # [64, N]
        sinT = rope[b].T[D:, :]
        cos2 = np.vstack([cosT, cosT]).astype(BF)                 # [128, N]
        sgn = np.where(np.arange(128) % 2 == 0, -1.0, 1.0)[:, None]
        sin2s = (np.vstack([sinT, sinT]) * sgn).astype(BF)        # [128, N]

        # qk weight rows ordered [q_h0..q_h3, k_h0..k_h3]
        qk_rows = []
        bqk_rows = []
        for h in heads:
            qk_rows.append(w_qkv[D * h:D * (h + 1), :])           # q rows
            bqk_rows.append(b_qkv[D * h:D * (h + 1)])
        for h in heads:
            qk_rows.append(w_qkv[C + D * h:C + D * (h + 1), :])   # k rows
            bqk_rows.append(b_qkv[C + D * h:C + D * (h + 1)])
        wqk = np.vstack(qk_rows)                                  # [512, C]
        wqkT = np.ascontiguousarray(wqk.T).astype(BF)             # [C, 512]
        wqk_r = np.ascontiguousarray(
            wqkT.reshape(KC, 128, 512).transpose(1, 0, 2)
            .reshape(128, KC * 512))
        bqk_v = np.ascontiguousarray(
            np.concatenate(bqk_rows).astype(np.float32).reshape(4, 128).T)

        h0 = HL * g
        wv = w_qkv[2 * C + D * h0:2 * C + D * h0 + CL, :]          # [256, C]
        wvT = np.ascontiguousarray(wv.T).astype(BF)                # [C, 256]
        wv_r = np.ascontiguousarray(
            wvT.reshape(KC, 128, CL).transpose(1, 0, 2)
            .reshape(128, KC * CL))

        # out-projection: this core owns output channels
        # [out_rows*g, out_rows*(g+1)).  The contraction rows are ordered
        # (head_local hl, rank r, d) to match the per-head AllGather layout
        # (gather hl concatenates rank blocks of 64 rows).
        osl = slice(out_rows * g, out_rows * (g + 1))
        wp_rows = []
        for hl in range(HL):
            for r in range(group_size):
                gh = HL * r + hl                   # global head of (r, hl)
                wp_rows.append(w_proj[osl, D * gh:D * (gh + 1)].T)  # [64,256]
        wprojT = np.ascontiguousarray(np.vstack(wp_rows)).astype(BF)  # [C,256]
        wproj_r = np.ascontiguousarray(
            wprojT.reshape(KC, 128, out_rows).transpose(1, 0, 2)
            .reshape(128, KC * out_rows))

        beff_shard = np.ascontiguousarray(
            b_eff[osl].astype(np.float32).reshape(2, 128).T)

        in_maps.append({
            "xT": x_r, "cos2": cos2, "sin2s": sin2s,
            "wqkT": wqk_r, "bqk": bqk_v, "wvT": wv_r,
            "wprojT": wproj_r, "beff": beff_shard,
        })
    return in_maps


def assemble(results, n_cores=N_CORES, group_size=4):
    out_rows = C // group_size
    out = np.empty((B, N, C), dtype=np.float32)
    for c in range(n_cores):
        b = (c // group_size) % B
        g = c % group_size
        outT_shard = results[c]["out"]                 # [out_rows, N] f32
        out[b, :, out_rows * g:out_rows * (g + 1)] = outT_shard.T
    return out


_NC_CACHE = {}


def _get_nc():
    if "nc" not in _NC_CACHE:
        _NC_CACHE["nc"] = build_kernel()
    return _NC_CACHE["nc"]


def _run(inputs, trace=False, tmpdir=None):
    nc = _get_nc()
    inputs = {k: np.asarray(v) for k, v in inputs.items()}
    in_maps = shard_inputs(**inputs)
    res = run_bass_kernel_spmd(nc, in_maps, core_ids=list(range(N_CORES)),
                               trace=trace, tmpdir=tmpdir)
    return assemble(res.results), res


def kernel(**inputs):
    out, _ = _run(inputs)
    return out


# revision 50
# speedup vs baseline: 1.0676x; 1.0108x over previous
"""Multi-head attention (B=2, N=2048, C=1024, H=16, D=64) on 8 TRN2 NeuronCores.

Sharding: core c = (batch b = c//4) x (head-group g = c%4 -> heads 4g..4g+3).
Data parallel on B, tensor parallel on heads.  After each head's softmax
normalization, the [64, cw] head-output is AllGathered (fp16) within the
4-core batch group; each core then runs the full-K (1024) out-projection
for its 256-channel output slice locally -- no reduce, no post-projection
collective, and the gathers overlap the attention pipeline.

The projections are software-pipelined into the attention stream: only the
k-projection (m2) and the first half of the q-projection (m0 @ n0,n1) run
before head 0 starts; the remaining qk projection chunks, the whole
v-projection and the previous chunk's out-projection quadrants are folded
into the attention pipeline's PE slack (the inner loop is ACT/exp-bound).

Everything on device stays transposed ([channel, position]); the host
pre-transposes inputs and post-transposes the output.
"""

import numpy as np

import concourse.bacc as bacc
import concourse.tile as tile
import concourse.mybir as mybir
from concourse.bass_utils import run_bass_kernel_spmd

B, N, C, H = 2, 2048, 1024, 16
D = C // H          # 64
HL = H // 4         # 4 heads per core
CL = HL * D         # 256 local channels
N_CORES = 8
GROUPS = [[0, 1, 2, 3], [4, 5, 6, 7]]

F32 = mybir.dt.float32
BF16 = mybir.dt.float16
BF = np.float16

KC = C // 128       # 8  K-chunks of the input channel dim
NI = N // 512       # 4  512-wide i-chunks
NJ = N // 128       # 16 128-row j-chunks


def build_kernel(n_cores=N_CORES, groups=GROUPS):
    group_size = len(groups[0])
    out_rows = C // group_size          # 256 output channels per core

    nc = bacc.Bacc("TRN2", target_bir_lowering=False, debug=False,
                   num_devices=n_cores)

    # all inputs are host-packed into 128-partition-major panels so each
    # loads in one full-bandwidth DMA (per-transfer setup dominates at
    # smaller sizes): x as (half, kc, 1024) columns, weights kc-major
    xT = nc.declare_dram_parameter("xT", [128, 2 * KC * 1024], BF16,
                                   isOutput=False)
    cos2 = nc.declare_dram_parameter("cos2", [128, N], BF16, isOutput=False)
    sin2s = nc.declare_dram_parameter("sin2s", [128, N], BF16, isOutput=False)
    wqkT = nc.declare_dram_parameter("wqkT", [128, KC * 512], BF16,
                                     isOutput=False)
    bqk = nc.declare_dram_parameter("bqk", [128, 4], F32, isOutput=False)
    wvT = nc.declare_dram_parameter("wvT", [128, KC * CL], BF16,
                                    isOutput=False)
    # out-projection weights, rows ordered (head_local, rank, d) to match
    # the AllGather concat order; cols = this core's 256 output channels
    wprojT = nc.declare_dram_parameter("wprojT", [128, KC * out_rows], BF16,
                                       isOutput=False)
    beff = nc.declare_dram_parameter("beff", [128, 2], F32, isOutput=False)
    out = nc.declare_dram_parameter("out", [out_rows, N], F32, isOutput=True)

    with tile.TileContext(nc) as tc:
        with tc.tile_pool(name="dram", bufs=1, space="DRAM") as dram, \
             tc.tile_pool(name="sbuf", bufs=1) as sb, \
             tc.tile_pool(name="psum", bufs=1, space="PSUM") as ps:

            # tile for clock-warming matmuls (see _warm_pe)
            warm = sb.tile([128, 128], BF16, name="warm", tag="warm")
            nc.vector.memset(warm[:], 0.001)

            def _warm_pe(tag, n):
                # short matmuls alternating two PSUM tiles: keeps the PE's
                # activity monitor busy so the clock gate stays at full rate
                wps = [ps.tile([128, 64], F32, name=f"warmp{tag}_{a}",
                               tag="sc", bufs=3) for a in range(2)]
                for r in range(n):
                    nc.tensor.matmul(wps[r % 2][:], warm[:], warm[:, :64],
                                     start=True, stop=True)

            _warm_pe("s", 16)

            # ---- input DMA: wqk first, then x in n-major column slices so
            # the kc-outer qk matmul chain for chunk n can start as soon as
            # that chunk's 8 column slices land ----
            # x is loaded in two [*, 1024] column halves (2KB lines keep the
            # DMA at full efficiency): the first half unblocks head 0's
            # whole q range and its first 8 score chunks; the second half
            # streams under head 0's pipeline.
            #
            # The "scalar" DMA queue is the ACT engine's instruction queue:
            # any trigger parked there blocks the evictions/exp stream
            # behind it (head-of-line, ring-gated by transfer completions).
            # So ACT only carries early-critical triggers; the bulk goes on
            # sync, and the second-half x triggers for ACT are emitted after
            # the pre-attention evictions (see below).
            bqk_sb = sb.tile([128, 4], F32, name="bqk_sb", tag="bqk")
            nc.sync.dma_start(bqk_sb[:], bqk.ap())
            wqk_t = sb.tile([128, KC * 512], BF16, name="wqk_t", tag="wqk")
            nc.scalar.dma_start(wqk_t[:], wqkT.ap())
            # first x half ahead of everything else (it gates the whole
            # front-end), quartered so the kc-outer qk chain can trickle
            x_sb = sb.tile([128, 2 * KC * 1024], BF16, name="x_sb", tag="x")
            for qtr in range(4):
                eng = nc.sync if qtr % 2 == 0 else nc.scalar
                eng.dma_start(x_sb[:, 2048 * qtr:2048 * (qtr + 1)],
                              xT.ap()[:, 2048 * qtr:2048 * (qtr + 1)])
            cos_sb = sb.tile([128, N], BF16, name="cos_sb", tag="cos_sb")
            sin_sb = sb.tile([128, N], BF16, name="sin_sb", tag="sin_sb")
            nc.sync.dma_start(cos_sb[:], cos2.ap())
            nc.scalar.dma_start(sin_sb[:], sin2s.ap())
            wv_t = sb.tile([128, KC * CL], BF16, name="wv_t", tag="wv")
            nc.sync.dma_start(wv_t[:], wvT.ap())
            nc.sync.dma_start(x_sb[:, 8192:12288], xT.ap()[:, 8192:12288])
            nc.scalar.dma_start(x_sb[:, 12288:16384], xT.ap()[:, 12288:16384])
            wproj_t = sb.tile([128, KC * out_rows], BF16, name="wproj_t",
                              tag="wproj")
            nc.sync.dma_start(wproj_t[:], wprojT.ap())
            beff_sb = sb.tile([128, 2], F32, name="beff_sb", tag="beff")
            nc.sync.dma_start(beff_sb[:], beff.ap())

            def xap(kc, c0, c1):
                # x panel columns: 8192*half + 1024*kc + (c % 1024)
                nn = c0 // 1024
                base = 8192 * nn + 1024 * kc + (c0 - 1024 * nn)
                return x_sb[:, base:base + (c1 - c0)]

            def wqk_ap(kc, m):
                return wqk_t[:, 512 * kc + 128 * m:512 * kc + 128 * (m + 1)]

            def wv_ap(kc):
                return wv_t[:, CL * kc:CL * (kc + 1)]

            def wproj_ap(kc, mc):
                return wproj_t[:, out_rows * kc + 128 * mc:
                               out_rows * kc + 128 * (mc + 1)]

            # warm up the collective path during the preamble so the first
            # real gather doesn't absorb the first-collective cost
            agw_in = dram.tile([64, 8], BF16, name="agw_in", tag="agw_in")
            agw_out = dram.tile([64 * group_size, 8], BF16, name="agw_out",
                                tag="agw_out")
            agw_sb = sb.tile([64, 8], BF16, name="agw_sb", tag="agw_sb")
            nc.vector.memset(agw_sb[:], 0.0)
            nc.sync.dma_start(agw_in[:], agw_sb[:])
            nc.gpsimd.collective_compute(
                "AllGather", mybir.AluOpType.bypass, replica_groups=groups,
                ins=[agw_in[:]], outs=[agw_out[:]])

            # ---- qk projection + RoPE, per (m, n-chunk) job ----
            # chunk m rows: m=0:[q_h0,q_h1] m=1:[q_h2,q_h3] m=2:[k_h0,k_h1] m=3:[k_h2,k_h3]
            # so q and k of head h sit at the same partition offset 64*(h%2).
            # k of each head lands in its own zero-padded [128, N] tile so the
            # scores matmul can contract over K=128 (16-bit matmuls run at
            # half rate for K=64 -- zero rows buy back the full rate).
            k_t = []      # 4 tiles: k_h at rows 64*(h%2), zeros elsewhere
            for h in range(4):
                kt = sb.tile([128, N], BF16, name=f"ktile{h}", tag=f"ktile{h}")
                z = slice(0, 64) if h % 2 == 1 else slice(64, 128)
                # gpsimd: these 1.8us memsets would clog the DVE queue
                # ahead of the RoPE chain that gates the first exp
                nc.gpsimd.memset(kt[z, :], 0.0)
                k_t.append(kt)
            swap_mask = [i ^ 1 for i in range(32)]
            qks_t = [sb.tile([128, N], BF16, name=f"qks{m}", tag=f"qks{m}")
                     for m in range(4)]
            q_r = [sb.tile([128, N], BF16, name=f"qkr{m}", tag=f"qkr{m}")
                   for m in range(2)]

            def qk_mm(nn, m):
                # accumulate one 1024-wide projection half over the 8 kc
                # chunks (single PSUM acc, two 512 column groups)
                acc = ps.tile([128, 1024], F32, name=f"qa{nn}_{m}", tag="sc",
                              bufs=3)
                for kc in range(KC):
                    for q in range(2):
                        nc.tensor.matmul(
                            acc[:, 512 * q:512 * (q + 1)],
                            wqk_ap(kc, m),
                            xap(kc, 1024 * nn + 512 * q,
                                1024 * nn + 512 * (q + 1)),
                            start=(kc == 0), stop=(kc == KC - 1))
                return acc

            def qk_fin(nn, m, acc, act=True):
                # bias-add eviction + RoPE.  When folded this runs two jc
                # slots after the matmuls so the ACT/DVE work never waits on
                # in-flight PE instructions (head-of-line on the exp stream).
                nsl = slice(1024 * nn, 1024 * (nn + 1))
                qks = qks_t[m]
                if act:
                    nc.scalar.activation(
                        qks[:, nsl], acc[:],
                        mybir.ActivationFunctionType.Identity,
                        bias=bqk_sb[:, m:m + 1])
                else:
                    nc.vector.tensor_scalar_add(qks[:, nsl], acc[:],
                                                bqk_sb[:, m:m + 1])
                # RoPE: qk' = qks*cos2 + shift(qks)*sin2s
                # (pair-swap of adjacent partitions via DVE stream shuffle)
                shf = sb.tile([128, 1024], BF16, name=f"shf{nn}_{m}",
                              tag="shf", bufs=2)
                nc.vector.stream_shuffle(shf[:], qks[:, nsl], swap_mask)
                t2 = sb.tile([128, 1024], BF16, name=f"rtmp{nn}_{m}",
                             tag="ropetmp", bufs=2)
                nc.vector.tensor_mul(t2[:], shf[:], sin_sb[:, nsl])
                if m < 2:
                    qkr = q_r[m]
                    nc.vector.tensor_mul(qkr[:, nsl], qks[:, nsl],
                                         cos_sb[:, nsl])
                    nc.vector.tensor_add(qkr[:, nsl], qkr[:, nsl], t2[:])
                else:
                    t1 = sb.tile([128, 1024], BF16, name=f"rtc{nn}_{m}",
                                 tag="ropetc", bufs=2)
                    nc.vector.tensor_mul(t1[:], qks[:, nsl], cos_sb[:, nsl])
                    h0, h1 = 2 * (m - 2), 2 * (m - 2) + 1
                    nc.vector.tensor_add(k_t[h0][0:64, nsl], t1[0:64, :],
                                         t2[0:64, :])
                    nc.vector.tensor_add(k_t[h1][64:128, nsl], t1[64:128, :],
                                         t2[64:128, :])

            def qk_job(nn, m, act=True):
                qk_fin(nn, m, qk_mm(nn, m), act)

            # pre-attention: k-proj and q-proj only for what head 0's first
            # 8 score chunks touch (first column half); the rest is folded
            # into the attention pipeline as the second x half lands
            qk_job(0, 2)
            qk_job(0, 0)

            # ---- v projection (natural [j, ch] layout, ones col per head) ----
            vaug = [None] * NJ

            def vproj_mm(jp):
                pvs = [ps.tile([128, CL], F32, name=f"pv{2*jp+a}", tag="sc",
                               bufs=3) for a in range(2)]
                for kc in range(KC):
                    for a in range(2):
                        jc = 2 * jp + a
                        nc.tensor.matmul(
                            pvs[a][:],
                            xap(kc, 128 * jc, 128 * (jc + 1)),
                            wv_ap(kc),
                            start=(kc == 0), stop=(kc == KC - 1))
                return pvs

            def vproj_fin(jp, pvs, act=True):
                for a in range(2):
                    jc = 2 * jp + a
                    va = sb.tile([128, HL * (D + 1)], BF16, name=f"vaug{jc}",
                                 tag=f"vaug{jc}")
                    nc.vector.memset(va[:, D::D + 1], 1.0)
                    if act:
                        nc.scalar.activation(
                            va.rearrange("p (h e) -> p h e", e=D + 1)[:, :, 0:D],
                            pvs[a].rearrange("p (h e) -> p h e", e=D)[:, :, :],
                            mybir.ActivationFunctionType.Copy)
                    else:
                        nc.vector.tensor_copy(
                            va.rearrange("p (h e) -> p h e", e=D + 1)[:, :, 0:D],
                            pvs[a].rearrange("p (h e) -> p h e", e=D)[:, :, :])
                    vaug[jc] = va

            # per-partition bias AP used to shift scores before fp16 exp
            eshift = sb.tile([128, 1], F32, name="eshift", tag="eshift")
            nc.vector.memset(eshift[:], -16.0)
            # K=1 ones row used to broadcast denominators across partitions
            ones64 = sb.tile([1, 64], BF16, name="ones64", tag="ones64")
            nc.vector.memset(ones64[:], 1.0)

            # ---- attention, per i-chunk; per-head AllGather of the
            # normalized output; remaining projections and the previous
            # chunk's out-projection folded into the head pipelines ----
            chunks = [(0, 1024), (1024, 1024)]
            n_chunks = len(chunks)

            ag_in = [[dram.tile([64, cw], BF16, name=f"agin{ih}_{hl}",
                                tag=f"agin{ih}_{hl}")
                      for hl in range(4)] for ih, (i0, cw) in enumerate(chunks)]
            ag_out = [[dram.tile([64 * group_size, cw], BF16,
                                 name=f"agout{ih}_{hl}", tag=f"agout{ih}_{hl}")
                       for hl in range(4)] for ih, (i0, cw) in enumerate(chunks)]

            # AGs fired so far, in order; readbacks are emitted two head-slots
            # after the AG fires so a pending readback on the gpsimd queue
            # never sits between two AG triggers (that would serialize the
            # collective stream: AG k+1 couldn't trigger until AG k finished)
            ag_fired = []
            rb_state = {"done": 0}

            def drain_readbacks(limit=1):
                while len(ag_fired) - rb_state["done"] > limit:
                    fih, fhl, fcw = ag_fired[rb_state["done"]]
                    emit_readback(fih, fhl, fcw)
                    rb_state["done"] += 1

            def finalize_head(ih, hl, oacc, cw):
                # normalize: o[:, i] / den[i].  Broadcast den across
                # partitions with a K=1 matmul, then reciprocal+mul on 64
                # partitions; gather the per-head result across the group.
                den = sb.tile([1, cw], BF16, name=f"den{ih}_{hl}",
                              tag="den", bufs=2)
                nc.vector.tensor_copy(den[:], oacc[64:65, :])
                rb = ps.tile([64, cw], F32, name=f"rb{ih}_{hl}",
                             tag="sc", bufs=3)
                for q in range(cw // 512):
                    nc.tensor.matmul(rb[:, 512 * q:512 * (q + 1)],
                                     ones64[:],
                                     den[:, 512 * q:512 * (q + 1)],
                                     start=True, stop=True)
                rr = sb.tile([64, cw], F32, name=f"rr{ih}_{hl}", tag="rr",
                             bufs=2)
                nc.vector.reciprocal_approx_fast(rr[:], rb[:])
                oh = sb.tile([64, cw], BF16, name=f"oh{ih}_{hl}", tag="oh",
                             bufs=2)
                nc.vector.tensor_mul(oh[:], oacc[0:64, :], rr[:])
                nc.sync.dma_start(ag_in[ih][hl][:], oh[:])
                nc.gpsimd.collective_compute(
                    "AllGather", mybir.AluOpType.bypass,
                    replica_groups=groups,
                    ins=[ag_in[ih][hl][:]],
                    outs=[ag_out[ih][hl][:]])
                ag_fired.append((ih, hl, cw))

            # gathered o readback tiles, kc = hl*2 + half (row order matches
            # wprojT's (head_local, rank, d) ordering)
            o_rb = [[None] * KC for _ in range(n_chunks)]

            def emit_readback(ih, hl, cw):
                for half in range(2):
                    t = sb.tile([128, cw], BF16, name=f"orb{ih}_{hl}_{half}",
                                tag="orb", bufs=12)
                    nc.gpsimd.dma_start(
                        t[:], ag_out[ih][hl][128 * half:128 * (half + 1), :])
                    o_rb[ih][2 * hl + half] = t

            def proj_mm(ih, mc, n2, i0, pp, kcs, start, stop):
                isl = slice(512 * n2, 512 * (n2 + 1))
                for kc in kcs:
                    nc.tensor.matmul(
                        pp[:],
                        wproj_ap(kc, mc),
                        o_rb[ih][kc][:, isl],
                        start=start and kc == kcs[0],
                        stop=stop and kc == kcs[-1])

            def proj_evict(ih, mc, n2, i0, pp):
                fo = sb.tile([128, 512], F32, name=f"fo{ih}_{mc}_{n2}",
                             tag="fo", bufs=4)
                nc.vector.tensor_scalar_add(fo[:], pp[:],
                                            beff_sb[:, mc:mc + 1])
                nc.sync.dma_start(
                    out.ap()[128 * mc:128 * (mc + 1),
                             i0 + 512 * n2:i0 + 512 * (n2 + 1)], fo[:])

            def emit_proj_job(ih, mc, n2, i0, cw):
                pp = ps.tile([128, 512], F32, name=f"pp{ih}_{mc}_{n2}",
                             tag="sc", bufs=3)
                proj_mm(ih, mc, n2, i0, pp, list(range(KC)), True, True)
                return pp

            def fold_proj(ih, hl, jc, job):
                fold2(ih, hl, jc, 2, lambda: emit_proj_job(*job),
                      lambda pp: proj_evict(job[0], job[1], job[2], job[3],
                                            pp))

            # fold schedule: (ih, hl, jc) -> list of thunks to emit inside
            # the attention pipeline at that point (PE slack).  Folded jobs
            # are two-stage: matmuls at jc, eviction/RoPE at jc+2, so the
            # ACT/DVE side never blocks the exp stream on in-flight matmuls
            folds = {}

            def add_fold(ih, hl, jc, fn):
                folds.setdefault((ih, hl, jc), []).append(fn)

            def fold2(ih, hl, jc, gap, mm_fn, fin_fn):
                # two-stage: matmuls at jc, eviction at jc+gap (so the
                # ACT/DVE side never waits on in-flight matmuls).  gap is
                # bounded by the sc-slot rotation: a fold's PSUM acc slot is
                # reassigned by the 3rd-next sc-tag tile, and the eviction
                # read must be emitted before that reassigning tile.
                box = {}
                add_fold(ih, hl, jc, lambda: box.__setitem__("a", mm_fn()))
                add_fold(ih, hl, min(jc + gap, NJ - 1),
                         lambda: fin_fn(box["a"]))

            def qk_mm_half(nn, m, q, acc=None):
                # one 512-wide half of a qk projection job (a full 3.4us job
                # inserted at a single jc overruns the ~2 buffered exps the
                # scores-ahead pipeline provides; halves fit)
                if acc is None:
                    acc = ps.tile([128, 1024], F32, name=f"qa{nn}_{m}",
                                  tag="sc", bufs=3)
                for kc in range(KC):
                    nc.tensor.matmul(
                        acc[:, 512 * q:512 * (q + 1)],
                        wqk_ap(kc, m),
                        xap(kc, 1024 * nn + 512 * q,
                            1024 * nn + 512 * (q + 1)),
                        start=(kc == 0), stop=(kc == KC - 1))
                return acc

            def fold_qk(ih, hl, jc, nn, m):
                box = {}
                add_fold(ih, hl, jc,
                         lambda: box.__setitem__("a", qk_mm_half(nn, m, 0)))
                add_fold(ih, hl, min(jc + 1, NJ - 1),
                         lambda: qk_mm_half(nn, m, 1, box["a"]))
                add_fold(ih, hl, min(jc + 2, NJ - 1),
                         lambda: qk_fin(nn, m, box["a"], act=True))

            def fold_vp(ih, hl, jc, jp):
                # two pv tiles: slot reassigned one creation earlier -> gap 1
                fold2(ih, hl, jc, 1, lambda: vproj_mm(jp),
                      lambda pvs: vproj_fin(jp, pvs, act=True))

            # the first half of the v-projection runs in the preamble
            # (the PE is DMA-gated idle there and wv/x arrive in time);
            # head 0 absorbs the second-half k-proj (m2 @ nn1, ahead of the
            # jc8+ groups that read those k columns) and the rest of the
            # v-projection; the second x half streams in underneath
            for jp in range(4):
                vproj_fin(jp, vproj_mm(jp), act=False)
            fold_qk(0, 0, 4, 1, 2)
            fold_vp(0, 0, 6, 4)
            fold_vp(0, 0, 8, 5)
            fold_vp(0, 0, 10, 6)
            fold_vp(0, 0, 12, 7)
            # k-proj heads 2,3 (m3): k_t[2] is read from head 2 on, so both
            # halves land in head 1.  q-proj m1 @ first half must be emitted
            # before head 2 (its scores read q_r[1][:, 0:1024] from jc0);
            # m1/m0 second halves before chunk 1 reads them.
            fold_qk(0, 1, 2, 0, 3)
            fold_qk(0, 1, 6, 1, 3)
            fold_qk(0, 1, 11, 0, 1)
            fold_qk(0, 2, 2, 1, 0)
            fold_qk(0, 3, 2, 1, 1)

            for ih, (i0, cw) in enumerate(chunks):
                ns = cw // 512
                # out-projection quadrants of the previous chunk, folded
                # into heads 2,3 (the last gather lands mid-head-1)
                if ih > 0:
                    pi0, pcw = chunks[ih - 1]
                    jobs = [(ih - 1, mc, n2, pi0, pcw)
                            for mc in range(out_rows // 128)
                            for n2 in range(pcw // 512)]
                    add_fold(ih, 2, 1,
                             lambda: drain_readbacks(limit=len(ag_fired) - 4))
                    fold_proj(ih, 2, 2, jobs[0])
                    fold_proj(ih, 2, 8, jobs[1])
                    fold_proj(ih, 3, 2, jobs[2])
                    fold_proj(ih, 3, 8, jobs[3])
                pending = None
                for hl in range(4):
                    qT = q_r[hl // 2]
                    kT = k_t[hl]
                    oacc = ps.tile([65, cw], F32, name=f"oacc{ih}_{hl}",
                                   tag="oacc", bufs=1)
                    exs = []

                    def emit_o(jc, oacc=oacc, exs=exs, hl=hl):
                        for q in range(ns):
                            nc.tensor.matmul(
                                oacc[:, 512 * q:512 * (q + 1)],
                                vaug[jc][:, (D + 1) * hl:(D + 1) * (hl + 1)],
                                exs[jc][:, 512 * q:512 * (q + 1)],
                                start=(jc == 0), stop=(jc == NJ - 1))

                    scs = {}

                    def emit_sc(jc, hl=hl, kT=kT, qT=qT, scs=scs):
                        sc = ps.tile([128, cw], F32, name=f"sc{ih}_{hl}_{jc}",
                                     tag="sc", bufs=3)
                        for q in range(ns):
                            nc.tensor.matmul(
                                sc[:, 512 * q:512 * (q + 1)],
                                kT[:, 128 * jc:128 * (jc + 1)],
                                qT[:, i0 + 512 * q:i0 + 512 * (q + 1)],
                                start=True, stop=True)
                        scs[jc] = sc

                    # scores run two j-chunks ahead of the exp stream so a
                    # folded projection never leaves ACT without buffered work
                    emit_sc(0)
                    emit_sc(1)
                    for jc in range(NJ):
                        ex = sb.tile([128, cw], BF16, name=f"ex{ih}_{hl}_{jc}",
                                     tag="ex", bufs=5)
                        # bias shifts all scores so fp16 exp can't overflow
                        # (softmax is shift-invariant, cancels in num/den)
                        nc.scalar.activation(ex[:], scs.pop(jc),
                                             mybir.ActivationFunctionType.Exp,
                                             scale=float(1.0 / np.sqrt(D)),
                                             bias=eshift[:])
                        exs.append(ex)
                        # software pipeline: the previous head's
                        # normalization chain is deferred into this head's
                        # pipeline so the exp stream never pauses at head
                        # boundaries
                        if jc == 1 and pending is not None:
                            finalize_head(*pending)
                            pending = None
                        # folds before the next score emission: a fold's
                        # finish stage reads a PSUM acc whose sc-tag slot is
                        # reassigned by the tile created three positions
                        # later -- the read must be emitted before that
                        for fn in folds.pop((ih, hl, jc), ()):
                            fn()
                        if jc + 2 < NJ:
                            emit_sc(jc + 2)
                        if jc >= 2:
                            # o-matmuls lag TWO j-chunks: the PE queue is
                            # in-order, so a shorter lag couples PE progress
                            # to the exp stream and leaves no buffered exp
                            # work when a folded projection occupies the PE
                            emit_o(jc - 2)
                    emit_o(NJ - 2)
                    emit_o(NJ - 1)
                    pending = (ih, hl, oacc, cw)
                    # emit readbacks for gathers fired three head-slots
                    # ago: by then the gather is long done, so the readback
                    # never parks on the gpsimd queue ahead of the next
                    # gather triggers (which would serialize the stream)
                    drain_readbacks(limit=2)
                # keep the PE clock-gate warm through the last head's
                # normalization chain so the tail starts at full rate
                _warm_pe(f"p{ih}", 12)
                if ih < n_chunks - 1:
                    finalize_head(*pending)
                    pending = None

            # ---- tail ----
            # finalize + gather the last head like any other, then the out-
            # projection quadrants: kc 0..5 (heads 0-2, gathers already
            # landed) run during the final gather; kc 6,7 finish after
            # the last gather's trigger must be enqueued before any
            # still-pending readback (a readback parked on the gpsimd queue
            # blocks triggers behind it until its own gather finishes)
            finalize_head(*pending)
            pending = None
            drain_readbacks(limit=0)
            li0, lcw = chunks[-1]
            tail_jobs = [(n_chunks - 1, mc, n2, li0, lcw)
                         for mc in range(out_rows // 128)
                         for n2 in range(lcw // 512)]
            pps = []
            for j, (ihx, mc, n2, i0, cw) in enumerate(tail_jobs):
                pp = ps.tile([128, 512], F32, name=f"tpp{mc}_{n2}",
                             tag="sc" if j < 3 else "oacc",
                             bufs=3 if j < 3 else 1)
                proj_mm(ihx, mc, n2, i0, pp, list(range(6)), True, False)
                pps.append(pp)
            for j, (ihx, mc, n2, i0, cw) in enumerate(tail_jobs):
                proj_mm(ihx, mc, n2, i0, pps[j], [6, 7], False, True)
                proj_evict(ihx, mc, n2, i0, pps[j])

    nc.compile()
    return nc


def shard_inputs(x, rope, w_qkv, b_qkv, w_proj, b_proj,
                 n_cores=N_CORES, group_size=4):
    """Per-core input maps. Host-side transposes/casts are part of sharding."""
    out_rows = C // group_size
    # fold the v-bias through the projection into an effective output bias
    b_v = b_qkv[2 * C:3 * C]
    b_eff = (b_proj + b_v @ w_proj.T).astype(np.float32)   # [C]

    in_maps = []
    for c in range(n_cores):
        b = (c // group_size) % B
        g = c % group_size
        heads = range(HL * g, HL * g + HL)

        xTb = np.ascontiguousarray(x[b].T).astype(BF)            # [C, N]
        # pack into the [128, (half, kc, 1024)] panel the kernel expects
        x_r = np.ascontiguousarray(
            xTb.reshape(KC, 128, 2, 1024).transpose(1, 2, 0, 3)
            .reshape(128, 2 * KC * 1024))

        cosT = rope[b].T[:D, :]                                   # [64, N]
        sinT = rope[b].T[D:, :]
        cos2 = np.vstack([cosT, cosT]).astype(BF)                 # [128, N]
        sgn = np.where(np.arange(128) % 2 == 0, -1.0, 1.0)[:, None]
        sin2s = (np.vstack([sinT, sinT]) * sgn).astype(BF)        # [128, N]

        # qk weight rows ordered [q_h0..q_h3, k_h0..k_h3]
        qk_rows = []
        bqk_rows = []
        for h in heads:
            qk_rows.append(w_qkv[D * h:D * (h + 1), :])           # q rows
            bqk_rows.append(b_qkv[D * h:D * (h + 1)])
        for h in heads:
            qk_rows.append(w_qkv[C + D * h:C + D * (h + 1), :])   # k rows
            bqk_rows.append(b_qkv[C + D * h:C + D * (h + 1)])
        wqk = np.vstack(qk_rows)                                  # [512, C]
        wqkT = np.ascontiguousarray(wqk.T).astype(BF)             # [C, 512]
        wqk_r = np.ascontiguousarray(
            wqkT.reshape(KC, 128, 512).transpose(1, 0, 2)
            .reshape(128, KC * 512))
        bqk_v = np.ascontiguousarray(
            np.concatenate(bqk_rows).astype(np.float32).reshape(4, 128).T)

        h0 = HL * g
        wv = w_qkv[2 * C + D * h0:2 * C + D * h0 + CL, :]          # [256, C]
        wvT = np.ascontiguousarray(wv.T).astype(BF)                # [C, 256]
        wv_r = np.ascontiguousarray(
            wvT.reshape(KC, 128, CL).transpose(1, 0, 2)
            .reshape(128, KC * CL))

        # out-projection: this core owns output channels
        # [out_rows*g, out_rows*(g+1)).  The contraction rows are ordered
        # (head_local hl, rank r, d) to match the per-head AllGather layout
        # (gather hl concatenates rank blocks of 64 rows).
        osl = slice(out_rows * g, out_rows * (g + 1))
        wp_rows = []
        for hl in range(HL):
            for r in range(group_size):
                gh = HL * r + hl                   # global head of (r, hl)
                wp_rows.append(w_proj[osl, D * gh:D * (gh + 1)].T)  # [64,256]
        wprojT = np.ascontiguousarray(np.vstack(wp_rows)).astype(BF)  # [C,256]
        wproj_r = np.ascontiguousarray(
            wprojT.reshape(KC, 128, out_rows).transpose(1, 0, 2)
            .reshape(128, KC * out_rows))

        beff_shard = np.ascontiguousarray(
            b_eff[osl].astype(np.float32).reshape(2, 128).T)

        in_maps.append({
            "xT": x_r, "cos2": cos2, "sin2s": sin2s,
            "wqkT": wqk_r, "bqk": bqk_v, "wvT": wv_r,
            "wprojT": wproj_r, "beff": beff_shard,
        })
    return in_maps


def assemble(results, n_cores=N_CORES, group_size=4):
    out_rows = C // group_size
    out = np.empty((B, N, C), dtype=np.float32)
    for c in range(n_cores):
        b = (c // group_size) % B
        g = c % group_size
        outT_shard = results[c]["out"]                 # [out_rows, N] f32
        out[b, :, out_rows * g:out_rows * (g + 1)] = outT_shard.T
    return out


_NC_CACHE = {}


def _get_nc():
    if "nc" not in _NC_CACHE:
        _NC_CACHE["nc"] = build_kernel()
    return _NC_CACHE["nc"]


def _run(inputs, trace=False, tmpdir=None):
    nc = _get_nc()
    inputs = {k: np.asarray(v) for k, v in inputs.items()}
    in_maps = shard_inputs(**inputs)
    res = run_bass_kernel_spmd(nc, in_maps, core_ids=list(range(N_CORES)),
                               trace=trace, tmpdir=tmpdir)
    return assemble(res.results), res


def kernel(**inputs):
    out, _ = _run(inputs)
    return out
